# revision 33
# baseline (speedup 1.0000x reference)
import numpy as np
from contextlib import ExitStack

B, V, J, P = 1024, 5023, 5, 36
NCORES = 8
BC = B // NCORES
PARENTS = np.array([0, 0, 1, 1, 1], dtype=np.int64)



def _rodrigues(rv, eps=1e-8):
    ang = np.linalg.norm(rv + eps, axis=1, keepdims=True)
    d = rv / ang
    cos = np.cos(ang)[:, :, None]
    sin = np.sin(ang)[:, :, None]
    rx, ry, rz = d[:, 0], d[:, 1], d[:, 2]
    z = np.zeros_like(rx)
    K = np.stack([z, -rz, ry, rz, z, -rx, -ry, rx, z], axis=1).reshape(-1, 3, 3)
    I = np.eye(3, dtype=rv.dtype)[None]
    return I + sin * K + (1.0 - cos) * (K @ K)


def _rot6d(x):
    a1, a2 = x[:, :3], x[:, 3:]
    b1 = a1 / np.linalg.norm(a1, axis=-1, keepdims=True)
    b2 = a2 - np.sum(b1 * a2, axis=-1, keepdims=True) * b1
    b2 = b2 / np.linalg.norm(b2, axis=-1, keepdims=True)
    b3 = np.cross(b1, b2)
    return np.stack([b1, b2, b3], axis=-2)


def _make_T(R, t):
    top = np.concatenate([R, t[..., None]], axis=-1)
    bot = np.broadcast_to(
        np.array([0.0, 0.0, 0.0, 1.0], R.dtype), top.shape[:-2] + (1, 4)
    )
    return np.concatenate([top, bot], axis=-2)


def host_prep(inputs):
    g6 = np.asarray(inputs["global_pose_params_6d"], np.float64)
    nk = np.asarray(inputs["neck_pose_params_ax"], np.float64)
    jw = np.asarray(inputs["jaw_pose_params_ax"], np.float64)
    ey = np.asarray(inputs["eye_pose_params_ax"], np.float64)
    jt = np.asarray(inputs["J_transformed_rest"], np.float64)

    Rg = _rot6d(g6)
    Rn = _rodrigues(nk)
    Rj = _rodrigues(jw)
    Rel = _rodrigues(ey[:, :3])
    Rer = _rodrigues(ey[:, 3:])
    rot_mats = np.stack([Rg, Rn, Rj, Rel, Rer], axis=1)

    rel = jt.copy()
    rel[:, 1:] -= jt[:, PARENTS[1:]]
    Tm = _make_T(rot_mats, rel)
    chain = [Tm[:, 0]]
    for i in range(1, J):
        chain.append(chain[int(PARENTS[i])] @ Tm[:, i])
    tr = np.stack(chain, axis=1)
    posed = tr[:, :, :3, 3]
    Rw = tr[:, :, :3, :3]
    t = posed - np.einsum("bjhw,bjw->bjh", Rw, jt)
    A = _make_T(Rw, t)

    A34 = np.ascontiguousarray(A[:, :, :3, :4], np.float32)
    PF = np.ascontiguousarray(
        (rot_mats[:, 1:5] - np.eye(3)).reshape(B, -1), np.float32
    )
    return A34, PF


def host_reference_emulation(inputs):
    A34, PF = host_prep(inputs)
    vs = np.asarray(inputs["v_shaped_expressed"], np.float32).reshape(B, V * 3)
    W = np.asarray(inputs["lbs_weights"], np.float32)
    pd = np.asarray(inputs["posedirs"], np.float32)
    PDt = pd.transpose(1, 0, 2).reshape(36, V * 3)
    pbs = PF @ PDt
    v = (vs + pbs).reshape(B, V, 3)
    T = np.einsum("bjhw,vj->bvhw", A34, W)
    out = np.einsum("bvhw,bvw->bvh", T[:, :, :, :3], v) + T[:, :, :, 3]
    return out.astype(np.float32)



SLAB = 1024
PAD = 8
CH = 256
NMAX = 512


def build_nc(bc=BC, v=V):
    import concourse.bacc as bacc
    import concourse.bass as bass_mod
    import concourse.tile as tile
    from concourse import mybir

    f32 = mybir.dt.float32
    f32r = mybir.dt.float32r
    vp = v + PAD

    nc = bacc.Bacc()
    vs_d = nc.dram_tensor("vs", [bc, v * 3], f32, kind="ExternalInput")
    wat_d = nc.dram_tensor("wat", [5, vp + 12 * bc], f32r, kind="ExternalInput")
    pfpd_d = nc.dram_tensor("pfpd", [36, bc + v * 3 + PAD], f32r, kind="ExternalInput")
    eye_d = nc.dram_tensor("eye", [bc, bc], f32r, kind="ExternalInput")
    out_d = nc.dram_tensor("out", [bc, v * 3], f32, kind="ExternalOutput")

    with tile.TileContext(nc) as tc, ExitStack() as ctx:
        singles = ctx.enter_context(tc.tile_pool(name="singles", bufs=1))
        sb_wat = singles.tile([5, vp + 12 * bc], f32r)
        nc.sync.dma_start(out=sb_wat, in_=wat_d[:])
        sb_pfpd = singles.tile([36, bc + v * 3 + PAD], f32r)
        nc.sync.dma_start(out=sb_pfpd, in_=pfpd_d[:])
        sb_eye = singles.tile([bc, bc], f32r)
        nc.sync.dma_start(out=sb_eye, in_=eye_d[:])
        sb_pf = sb_pfpd[:, :bc]
        wt = sb_wat[:, :vp]

        def at_slice(h, w):
            o = vp + (h * 4 + w) * bc
            return sb_wat[:, o : o + bc]

        vs_pool = ctx.enter_context(tc.tile_pool(name="vsp", bufs=2))
        out_pool = ctx.enter_context(tc.tile_pool(name="outp", bufs=2))
        v_pool = ctx.enter_context(tc.tile_pool(name="vv", bufs=3))
        m_pool = ctx.enter_context(tc.tile_pool(name="mm", bufs=2))
        pP = ctx.enter_context(tc.tile_pool(name="pP", bufs=1, space="PSUM"))
        pO = ctx.enter_context(tc.tile_pool(name="pO", bufs=1, space="PSUM"))


        for s0 in range(0, v, SLAB):
            sv = min(SLAB, v - s0)
            vs_t = vs_pool.tile([bc, sv * 3], f32, tag="vs")
            nc.sync.dma_start(out=vs_t, in_=vs_d[:, s0 * 3 : (s0 + sv) * 3])
            out_t = out_pool.tile([bc, sv * 3], f32, tag="out")
            out3 = out_t[:].rearrange("p (a c) -> p a c", c=3)

            for c0 in range(s0, s0 + sv, CH):
                cv = min(CH, s0 + sv - c0)
                co = c0 - s0
                cvp = cv + (cv & 1)

                P = pP.tile([bc, 12 * CH], f32, tag="P")
                np_ = CH if cvp == CH else cvp
                for h in range(3):
                    for w in range(3):
                        o = (h * 3 + w) * CH
                        nc.tensor.matmul(
                            P[:, o : o + np_],
                            lhsT=at_slice(h, w),
                            rhs=wt[:, c0 : c0 + np_],
                            start=True,
                            stop=True,
                        )
                pb0 = 9 * CH
                for n0, nn in ((0, CH), (CH, 2 * CH)):
                    nn = min(nn, cv * 3 - n0)
                    if nn <= 0:
                        break
                    nn += nn & 1
                    nc.tensor.matmul(
                        P[:, pb0 + n0 : pb0 + n0 + nn],
                        lhsT=sb_pf,
                        rhs=sb_pfpd[:, bc + c0 * 3 + n0 : bc + c0 * 3 + n0 + nn],
                        start=True,
                        stop=True,
                    )

                v_t = v_pool.tile([bc, cv * 3], f32, tag="v")
                nc.vector.tensor_add(
                    v_t[:],
                    vs_t[:, co * 3 : (co + cv) * 3],
                    P[:, pb0 : pb0 + cv * 3],
                )

                M = m_pool.tile([bc, 3, 3, CH], f32r, tag="M")
                t9v = P[:, : 9 * CH].rearrange("p (a b c) -> p a b c", b=3, c=CH)
                vt_ap = v_t[:]
                vb = bass_mod.AP(
                    tensor=vt_ap.tensor,
                    offset=vt_ap.offset,
                    ap=[list(vt_ap.ap[0]), [0, 3], [1, 3], [3, cv]],
                )
                nc.vector.tensor_tensor(
                    M[:, :, :, :cv],
                    t9v[:, :, :, :cv],
                    vb,
                    op=mybir.AluOpType.mult,
                )

                O = pO.tile([bc, 3, CH], f32, tag="O")
                for d in range(3):
                    nc.tensor.matmul(
                        O[:, d, :np_],
                        lhsT=at_slice(d, 3),
                        rhs=wt[:, c0 : c0 + np_],
                        start=True,
                        stop=False,
                    )
                    for w in range(3):
                        nc.tensor.matmul(
                            O[:, d, :np_],
                            lhsT=sb_eye[:],
                            rhs=M[:, d, w, :np_],
                            start=False,
                            stop=(w == 2),
                        )

                nc.scalar.copy(
                    out3[:, co : co + cv, :],
                    O[:].rearrange("p a c -> p c a")[:, :cv, :],
                )

            nc.sync.dma_start(out=out_d[:, s0 * 3 : (s0 + sv) * 3], in_=out_t[:])

    _strip_matmul_self_waits(nc)
    if not nc.is_finalized():
        nc.finalize()
    return nc


def _strip_matmul_self_waits(nc):
    fn = nc.m.functions[0]
    pe_sems = set()
    for b in fn.blocks:
        for i in b.instructions:
            if i.opcode == "Matmult":
                for u in i.sync_info.on_update:
                    if u.ant_name.startswith("PE"):
                        pe_sems.add(u.ant_name)
    for b in fn.blocks:
        for i in b.instructions:
            if i.opcode != "Matmult":
                continue
            si = i.sync_info
            kept = [w for w in si.on_wait if w.ant_name not in pe_sems]
            if len(kept) != len(si.on_wait):
                si.on_wait = kept
                i.sync_info = si



_BUILT = {}


def _get_nc():
    if "nc" not in _BUILT:
        _BUILT["nc"] = build_nc()
    return _BUILT["nc"]


def make_in_maps(inputs):
    A34, PF = host_prep(inputs)
    vs = np.ascontiguousarray(
        np.asarray(inputs["v_shaped_expressed"], np.float32).reshape(B, V * 3)
    )
    W = np.asarray(inputs["lbs_weights"], np.float32)
    pd = np.asarray(inputs["posedirs"], np.float32)
    Wt = np.ascontiguousarray(W.T)
    PDt = np.ascontiguousarray(pd.transpose(1, 0, 2).reshape(36, V * 3))
    PFt = np.ascontiguousarray(PF.T)

    eye = np.eye(BC, dtype=np.float32)
    pad5 = np.zeros((5, PAD), np.float32)
    pad36 = np.zeros((36, PAD), np.float32)
    Wtp = np.concatenate([Wt, pad5], axis=1)

    in_maps = []
    for c in range(NCORES):
        sl = slice(c * BC, (c + 1) * BC)
        AT_c = A34[sl].transpose(1, 2, 3, 0).reshape(5, 12 * BC)
        wat = np.ascontiguousarray(np.concatenate([Wtp, AT_c], axis=1))
        pfpd = np.ascontiguousarray(
            np.concatenate([PFt[:, sl], PDt, pad36], axis=1)
        )
        in_maps.append(
            {
                "vs": np.ascontiguousarray(vs[sl]),
                "wat": wat,
                "pfpd": pfpd,
                "eye": eye,
            }
        )
    return in_maps


def _enable_ldw_opt():
    import concourse.bass_utils as bu

    if getattr(bu, "_ldw_patched", False):
        return
    orig = bu.run_command

    def run_command_ldw(argv, **kw):
        argv = [
            "--enable-ldw-opt=true" if a == "--enable-ldw-opt=false" else a
            for a in argv
        ]
        return orig(argv, **kw)

    bu.run_command = run_command_ldw
    bu._ldw_patched = True


def run_on_device(inputs, trace=False):
    from concourse.bass_utils import run_bass_kernel_spmd

    _enable_ldw_opt()
    nc = _get_nc()
    in_maps = make_in_maps(inputs)
    res = run_bass_kernel_spmd(nc, in_maps, list(range(NCORES)), trace=trace)
    out = np.concatenate([res.results[i]["out"] for i in range(NCORES)], axis=0)
    return out.reshape(B, V, 3).astype(np.float32), res


def kernel(**inputs):
    out, _ = run_on_device(inputs, trace=False)
    return out


# revision 34
# speedup vs baseline: 1.2781x; 1.2781x over previous
import numpy as np
from contextlib import ExitStack

B, V, J, P = 1024, 5023, 5, 36
NCORES = 8
BC = B // NCORES
PARENTS = np.array([0, 0, 1, 1, 1], dtype=np.int64)



def _rodrigues(rv, eps=1e-8):
    ang = np.linalg.norm(rv + eps, axis=1, keepdims=True)
    d = rv / ang
    cos = np.cos(ang)[:, :, None]
    sin = np.sin(ang)[:, :, None]
    rx, ry, rz = d[:, 0], d[:, 1], d[:, 2]
    z = np.zeros_like(rx)
    K = np.stack([z, -rz, ry, rz, z, -rx, -ry, rx, z], axis=1).reshape(-1, 3, 3)
    I = np.eye(3, dtype=rv.dtype)[None]
    return I + sin * K + (1.0 - cos) * (K @ K)


def _rot6d(x):
    a1, a2 = x[:, :3], x[:, 3:]
    b1 = a1 / np.linalg.norm(a1, axis=-1, keepdims=True)
    b2 = a2 - np.sum(b1 * a2, axis=-1, keepdims=True) * b1
    b2 = b2 / np.linalg.norm(b2, axis=-1, keepdims=True)
    b3 = np.cross(b1, b2)
    return np.stack([b1, b2, b3], axis=-2)


def _make_T(R, t):
    top = np.concatenate([R, t[..., None]], axis=-1)
    bot = np.broadcast_to(
        np.array([0.0, 0.0, 0.0, 1.0], R.dtype), top.shape[:-2] + (1, 4)
    )
    return np.concatenate([top, bot], axis=-2)


def host_prep(inputs):
    g6 = np.asarray(inputs["global_pose_params_6d"], np.float64)
    nk = np.asarray(inputs["neck_pose_params_ax"], np.float64)
    jw = np.asarray(inputs["jaw_pose_params_ax"], np.float64)
    ey = np.asarray(inputs["eye_pose_params_ax"], np.float64)
    jt = np.asarray(inputs["J_transformed_rest"], np.float64)

    Rg = _rot6d(g6)
    Rn = _rodrigues(nk)
    Rj = _rodrigues(jw)
    Rel = _rodrigues(ey[:, :3])
    Rer = _rodrigues(ey[:, 3:])
    rot_mats = np.stack([Rg, Rn, Rj, Rel, Rer], axis=1)

    rel = jt.copy()
    rel[:, 1:] -= jt[:, PARENTS[1:]]
    Tm = _make_T(rot_mats, rel)
    chain = [Tm[:, 0]]
    for i in range(1, J):
        chain.append(chain[int(PARENTS[i])] @ Tm[:, i])
    tr = np.stack(chain, axis=1)
    posed = tr[:, :, :3, 3]
    Rw = tr[:, :, :3, :3]
    t = posed - np.einsum("bjhw,bjw->bjh", Rw, jt)
    A = _make_T(Rw, t)

    A34 = np.ascontiguousarray(A[:, :, :3, :4], np.float32)
    PF = np.ascontiguousarray(
        (rot_mats[:, 1:5] - np.eye(3)).reshape(B, -1), np.float32
    )
    return A34, PF


def host_reference_emulation(inputs):
    A34, PF = host_prep(inputs)
    vs = np.asarray(inputs["v_shaped_expressed"], np.float32).reshape(B, V * 3)
    W = np.asarray(inputs["lbs_weights"], np.float32)
    pd = np.asarray(inputs["posedirs"], np.float32)
    PDt = pd.transpose(1, 0, 2).reshape(36, V * 3)
    pbs = PF @ PDt
    v = (vs + pbs).reshape(B, V, 3)
    T = np.einsum("bjhw,vj->bvhw", A34, W)
    out = np.einsum("bvhw,bvw->bvh", T[:, :, :, :3], v) + T[:, :, :, 3]
    return out.astype(np.float32)



SLAB = 1024
PAD = 8
CH = 256
NMAX = 512


def build_nc(bc=BC, v=V):
    import concourse.bacc as bacc
    import concourse.bass as bass_mod
    import concourse.tile as tile
    from concourse import mybir

    f32 = mybir.dt.float32
    f32r = mybir.dt.float32r
    vp = v + PAD

    nc = bacc.Bacc()
    vs_d = nc.dram_tensor("vs", [bc, v * 3], f32, kind="ExternalInput")
    wat_d = nc.dram_tensor("wat", [5, vp + 12 * bc], f32r, kind="ExternalInput")
    pfpd_d = nc.dram_tensor("pfpd", [36, bc + v * 3 + PAD], f32r, kind="ExternalInput")
    eye_d = nc.dram_tensor("eye", [bc, bc], f32r, kind="ExternalInput")
    out_d = nc.dram_tensor("out", [bc, v * 3], f32, kind="ExternalOutput")

    with tile.TileContext(nc) as tc, ExitStack() as ctx:
        singles = ctx.enter_context(tc.tile_pool(name="singles", bufs=1))
        sb_wat = singles.tile([5, vp + 12 * bc], f32r)
        nc.sync.dma_start(out=sb_wat, in_=wat_d[:])
        sb_pfpd = singles.tile([36, bc + v * 3 + PAD], f32r)
        nc.sync.dma_start(out=sb_pfpd, in_=pfpd_d[:])
        sb_eye = singles.tile([bc, bc], f32r)
        nc.sync.dma_start(out=sb_eye, in_=eye_d[:])
        sb_pf = sb_pfpd[:, :bc]
        wt = sb_wat[:, :vp]

        def at_slice(h, w):
            o = vp + (h * 4 + w) * bc
            return sb_wat[:, o : o + bc]

        vs_pool = ctx.enter_context(tc.tile_pool(name="vsp", bufs=2))
        out_pool = ctx.enter_context(tc.tile_pool(name="outp", bufs=2))
        v_pool = ctx.enter_context(tc.tile_pool(name="vv", bufs=3))
        m_pool = ctx.enter_context(tc.tile_pool(name="mm", bufs=2))
        pP = ctx.enter_context(tc.tile_pool(name="pP", bufs=1, space="PSUM"))
        pO = ctx.enter_context(tc.tile_pool(name="pO", bufs=1, space="PSUM"))


        chunks = []
        for s0 in range(0, v, SLAB):
            sv = min(SLAB, v - s0)
            for c0 in range(s0, s0 + sv, CH):
                cv = min(CH, s0 + sv - c0)
                last_in_slab = c0 + cv == s0 + sv
                chunks.append((s0, sv, c0, cv, last_in_slab))

        slab_tiles = {}

        def o_block(st):
            (out3_p, M_p, c0_p, cv_p, co_p, s0_p, sv_p, last_p) = st
            np_p = CH if cv_p == CH else cv_p + (cv_p & 1)
            O = pO.tile([bc, 3, CH], f32, tag="O")
            for d in range(3):
                nc.tensor.matmul(
                    O[:, d, :np_p],
                    lhsT=at_slice(d, 3),
                    rhs=wt[:, c0_p : c0_p + np_p],
                    start=True,
                    stop=False,
                )
                for w in range(3):
                    nc.tensor.matmul(
                        O[:, d, :np_p],
                        lhsT=sb_eye[:],
                        rhs=M_p[:, d, w, :np_p],
                        start=False,
                        stop=(w == 2),
                    )
            nc.scalar.copy(
                out3_p[:, co_p : co_p + cv_p, :],
                O[:].rearrange("p a c -> p c a")[:, :cv_p, :],
            )
            if last_p:
                out_t_p = slab_tiles.pop(s0_p)
                nc.sync.dma_start(
                    out=out_d[:, s0_p * 3 : (s0_p + sv_p) * 3], in_=out_t_p[:]
                )

        pending = None
        for s0, sv, c0, cv, last_in_slab in chunks:
            if s0 not in slab_tiles:
                vs_t = vs_pool.tile([bc, sv * 3], f32, tag="vs")
                nc.sync.dma_start(out=vs_t, in_=vs_d[:, s0 * 3 : (s0 + sv) * 3])
                out_t = out_pool.tile([bc, sv * 3], f32, tag="out")
                slab_tiles[s0] = out_t
                cur_vs, cur_out3 = vs_t, out_t[:].rearrange("p (a c) -> p a c", c=3)
            co = c0 - s0
            cvp = cv + (cv & 1)
            np_ = CH if cvp == CH else cvp

            P = pP.tile([bc, 12 * CH], f32, tag="P")
            for h in range(3):
                for w in range(3):
                    o = (h * 3 + w) * CH
                    nc.tensor.matmul(
                        P[:, o : o + np_],
                        lhsT=at_slice(h, w),
                        rhs=wt[:, c0 : c0 + np_],
                        start=True,
                        stop=True,
                    )
            pb0 = 9 * CH
            for n0, nn in ((0, CH), (CH, 2 * CH)):
                nn = min(nn, cv * 3 - n0)
                if nn <= 0:
                    break
                nn += nn & 1
                nc.tensor.matmul(
                    P[:, pb0 + n0 : pb0 + n0 + nn],
                    lhsT=sb_pf,
                    rhs=sb_pfpd[:, bc + c0 * 3 + n0 : bc + c0 * 3 + n0 + nn],
                    start=True,
                    stop=True,
                )

            if pending is not None:
                o_block(pending)

            v_t = v_pool.tile([bc, cv * 3], f32, tag="v")
            nc.vector.tensor_add(
                v_t[:],
                cur_vs[:, co * 3 : (co + cv) * 3],
                P[:, pb0 : pb0 + cv * 3],
            )
            M = m_pool.tile([bc, 3, 3, CH], f32r, tag="M")
            t9v = P[:, : 9 * CH].rearrange("p (a b c) -> p a b c", b=3, c=CH)
            vt_ap = v_t[:]
            vb = bass_mod.AP(
                tensor=vt_ap.tensor,
                offset=vt_ap.offset,
                ap=[list(vt_ap.ap[0]), [0, 3], [1, 3], [3, cv]],
            )
            nc.vector.tensor_tensor(
                M[:, :, :, :cv],
                t9v[:, :, :, :cv],
                vb,
                op=mybir.AluOpType.mult,
            )
            pending = (cur_out3, M, c0, cv, co, s0, sv, last_in_slab)

        o_block(pending)

    _strip_matmul_self_waits(nc)
    if not nc.is_finalized():
        nc.finalize()
    return nc


def _strip_matmul_self_waits(nc):
    fn = nc.m.functions[0]
    pe_sems = set()
    for b in fn.blocks:
        for i in b.instructions:
            if i.opcode == "Matmult":
                for u in i.sync_info.on_update:
                    if u.ant_name.startswith("PE"):
                        pe_sems.add(u.ant_name)
    for b in fn.blocks:
        for i in b.instructions:
            if i.opcode != "Matmult":
                continue
            si = i.sync_info
            kept = [w for w in si.on_wait if w.ant_name not in pe_sems]
            if len(kept) != len(si.on_wait):
                si.on_wait = kept
                i.sync_info = si



_BUILT = {}


def _get_nc():
    if "nc" not in _BUILT:
        _BUILT["nc"] = build_nc()
    return _BUILT["nc"]


def make_in_maps(inputs):
    A34, PF = host_prep(inputs)
    vs = np.ascontiguousarray(
        np.asarray(inputs["v_shaped_expressed"], np.float32).reshape(B, V * 3)
    )
    W = np.asarray(inputs["lbs_weights"], np.float32)
    pd = np.asarray(inputs["posedirs"], np.float32)
    Wt = np.ascontiguousarray(W.T)
    PDt = np.ascontiguousarray(pd.transpose(1, 0, 2).reshape(36, V * 3))
    PFt = np.ascontiguousarray(PF.T)

    eye = np.eye(BC, dtype=np.float32)
    pad5 = np.zeros((5, PAD), np.float32)
    pad36 = np.zeros((36, PAD), np.float32)
    Wtp = np.concatenate([Wt, pad5], axis=1)

    in_maps = []
    for c in range(NCORES):
        sl = slice(c * BC, (c + 1) * BC)
        AT_c = A34[sl].transpose(1, 2, 3, 0).reshape(5, 12 * BC)
        wat = np.ascontiguousarray(np.concatenate([Wtp, AT_c], axis=1))
        pfpd = np.ascontiguousarray(
            np.concatenate([PFt[:, sl], PDt, pad36], axis=1)
        )
        in_maps.append(
            {
                "vs": np.ascontiguousarray(vs[sl]),
                "wat": wat,
                "pfpd": pfpd,
                "eye": eye,
            }
        )
    return in_maps


def _enable_ldw_opt():
    import concourse.bass_utils as bu

    if getattr(bu, "_ldw_patched", False):
        return
    orig = bu.run_command

    def run_command_ldw(argv, **kw):
        argv = [
            "--enable-ldw-opt=true" if a == "--enable-ldw-opt=false" else a
            for a in argv
        ]
        return orig(argv, **kw)

    bu.run_command = run_command_ldw
    bu._ldw_patched = True


def run_on_device(inputs, trace=False):
    from concourse.bass_utils import run_bass_kernel_spmd

    _enable_ldw_opt()
    nc = _get_nc()
    in_maps = make_in_maps(inputs)
    res = run_bass_kernel_spmd(nc, in_maps, list(range(NCORES)), trace=trace)
    out = np.concatenate([res.results[i]["out"] for i in range(NCORES)], axis=0)
    return out.reshape(B, V, 3).astype(np.float32), res


def kernel(**inputs):
    out, _ = run_on_device(inputs, trace=False)
    return out


# revision 38
# speedup vs baseline: 1.4126x; 1.1052x over previous
import numpy as np
from contextlib import ExitStack

B, V, J, P = 1024, 5023, 5, 36
NCORES = 8
BC = B // NCORES
PARENTS = np.array([0, 0, 1, 1, 1], dtype=np.int64)



def _rodrigues(rv, eps=1e-8):
    ang = np.linalg.norm(rv + eps, axis=1, keepdims=True)
    d = rv / ang
    cos = np.cos(ang)[:, :, None]
    sin = np.sin(ang)[:, :, None]
    rx, ry, rz = d[:, 0], d[:, 1], d[:, 2]
    z = np.zeros_like(rx)
    K = np.stack([z, -rz, ry, rz, z, -rx, -ry, rx, z], axis=1).reshape(-1, 3, 3)
    I = np.eye(3, dtype=rv.dtype)[None]
    return I + sin * K + (1.0 - cos) * (K @ K)


def _rot6d(x):
    a1, a2 = x[:, :3], x[:, 3:]
    b1 = a1 / np.linalg.norm(a1, axis=-1, keepdims=True)
    b2 = a2 - np.sum(b1 * a2, axis=-1, keepdims=True) * b1
    b2 = b2 / np.linalg.norm(b2, axis=-1, keepdims=True)
    b3 = np.cross(b1, b2)
    return np.stack([b1, b2, b3], axis=-2)


def _make_T(R, t):
    top = np.concatenate([R, t[..., None]], axis=-1)
    bot = np.broadcast_to(
        np.array([0.0, 0.0, 0.0, 1.0], R.dtype), top.shape[:-2] + (1, 4)
    )
    return np.concatenate([top, bot], axis=-2)


def host_prep(inputs):
    g6 = np.asarray(inputs["global_pose_params_6d"], np.float64)
    nk = np.asarray(inputs["neck_pose_params_ax"], np.float64)
    jw = np.asarray(inputs["jaw_pose_params_ax"], np.float64)
    ey = np.asarray(inputs["eye_pose_params_ax"], np.float64)
    jt = np.asarray(inputs["J_transformed_rest"], np.float64)

    Rg = _rot6d(g6)
    Rn = _rodrigues(nk)
    Rj = _rodrigues(jw)
    Rel = _rodrigues(ey[:, :3])
    Rer = _rodrigues(ey[:, 3:])
    rot_mats = np.stack([Rg, Rn, Rj, Rel, Rer], axis=1)

    rel = jt.copy()
    rel[:, 1:] -= jt[:, PARENTS[1:]]
    Tm = _make_T(rot_mats, rel)
    chain = [Tm[:, 0]]
    for i in range(1, J):
        chain.append(chain[int(PARENTS[i])] @ Tm[:, i])
    tr = np.stack(chain, axis=1)
    posed = tr[:, :, :3, 3]
    Rw = tr[:, :, :3, :3]
    t = posed - np.einsum("bjhw,bjw->bjh", Rw, jt)
    A = _make_T(Rw, t)

    A34 = np.ascontiguousarray(A[:, :, :3, :4], np.float32)
    PF = np.ascontiguousarray(
        (rot_mats[:, 1:5] - np.eye(3)).reshape(B, -1), np.float32
    )
    return A34, PF


def host_reference_emulation(inputs):
    A34, PF = host_prep(inputs)
    vs = np.asarray(inputs["v_shaped_expressed"], np.float32).reshape(B, V * 3)
    W = np.asarray(inputs["lbs_weights"], np.float32)
    pd = np.asarray(inputs["posedirs"], np.float32)
    PDt = pd.transpose(1, 0, 2).reshape(36, V * 3)
    pbs = PF @ PDt
    v = (vs + pbs).reshape(B, V, 3)
    T = np.einsum("bjhw,vj->bvhw", A34, W)
    out = np.einsum("bvhw,bvw->bvh", T[:, :, :, :3], v) + T[:, :, :, 3]
    return out.astype(np.float32)



SLAB = 1024
PAD = 8
CH = 256
NMAX = 512


def build_nc(bc=BC, v=V):
    import concourse.bacc as bacc
    import concourse.bass as bass_mod
    import concourse.tile as tile
    from concourse import mybir

    f32 = mybir.dt.float32
    f32r = mybir.dt.float32r

    nc = bacc.Bacc()
    vs_d = nc.dram_tensor("vs", [bc, v * 3], f32, kind="ExternalInput")
    wat_d = nc.dram_tensor("wat", [5, v + PAD + 12 * bc], f32r, kind="ExternalInput")
    pfpd_d = nc.dram_tensor("pfpd", [36, bc + v * 3 + PAD], f32r, kind="ExternalInput")
    out_d = nc.dram_tensor("out", [bc, v * 3], f32, kind="ExternalOutput")

    with tile.TileContext(nc) as tc, ExitStack() as ctx:
        singles = ctx.enter_context(tc.tile_pool(name="singles", bufs=1))
        sb_wat = singles.tile([5, v + PAD + 12 * bc], f32r)
        nc.sync.dma_start(out=sb_wat, in_=wat_d[:])
        sb_pfpd = singles.tile([36, bc + v * 3 + PAD], f32r)
        nc.sync.dma_start(out=sb_pfpd, in_=pfpd_d[:])
        sb_pf = sb_pfpd[:, :bc]

        vs_pool = ctx.enter_context(tc.tile_pool(name="vsp", bufs=2))
        out_pool = ctx.enter_context(tc.tile_pool(name="outp", bufs=2))
        t_pool = ctx.enter_context(tc.tile_pool(name="tsb", bufs=3))
        v_pool = ctx.enter_context(tc.tile_pool(name="vv", bufs=3))
        m_pool = ctx.enter_context(tc.tile_pool(name="mm", bufs=4))
        ppbs = ctx.enter_context(tc.tile_pool(name="ppbs", bufs=2, space="PSUM"))
        pT = ctx.enter_context(tc.tile_pool(name="pT", bufs=2, space="PSUM"))

        for s0 in range(0, v, SLAB):
            sv = min(SLAB, v - s0)
            vs_t = vs_pool.tile([bc, sv * 3], f32, tag="vs")
            nc.sync.dma_start(out=vs_t, in_=vs_d[:, s0 * 3 : (s0 + sv) * 3])
            out_t = out_pool.tile([bc, sv * 3], f32, tag="out")
            out3 = out_t[:].rearrange("p (a c) -> p a c", c=3)

            for c0 in range(s0, s0 + sv, CH):
                cv = min(CH, s0 + sv - c0)
                co = c0 - s0

                pbs_full = ppbs.tile([bc, CH * 3], f32, tag="pbs")
                pbs = pbs_full[:, : cv * 3]
                for n0 in range(0, cv * 3, NMAX):
                    nn = min(NMAX, cv * 3 - n0)
                    nn += nn & 1
                    nc.tensor.matmul(
                        pbs_full[:, n0 : n0 + nn],
                        lhsT=sb_pf,
                        rhs=sb_pfpd[
                            :, bc + c0 * 3 + n0 : bc + c0 * 3 + n0 + nn
                        ],
                        start=True,
                        stop=True,
                    )

                v_t = v_pool.tile([bc, cv * 3], f32, tag="v")
                nc.vector.tensor_add(
                    v_t[:], vs_t[:, co * 3 : (co + cv) * 3], pbs[:]
                )
                v3 = v_t[:].rearrange("p (a c) -> p a c", c=3)

                for h in range(3):
                    Tp = pT.tile([bc, 4, CH], f32, tag="T")
                    for w in range(4):
                        hw = h * 4 + w
                        cvp = cv + (cv & 1)
                        nc.tensor.matmul(
                            Tp[:, w, :cvp],
                            lhsT=sb_wat[:, v + PAD + hw * bc : v + PAD + (hw + 1) * bc],
                            rhs=sb_wat[:, c0 : c0 + cvp],
                            start=True,
                            stop=True,
                        )
                    T_sb = t_pool.tile([bc, 4, cv], f32, tag="tsb")
                    nc.scalar.copy(T_sb[:], Tp[:, :, :cv])

                    m = m_pool.tile([bc, 3, cv], f32, tag="m")
                    vt_ap = v_t[:]
                    vb = bass_mod.AP(
                        tensor=vt_ap.tensor,
                        offset=vt_ap.offset,
                        ap=[list(vt_ap.ap[0]), [1, 3], [3, cv]],
                    )
                    nc.vector.tensor_tensor(
                        m[:], T_sb[:, :3, :], vb, op=mybir.AluOpType.mult
                    )
                    s01 = m_pool.tile([bc, cv], f32, tag="s01")
                    s2 = m_pool.tile([bc, cv], f32, tag="s2")
                    nc.vector.tensor_add(s01[:], m[:, 0, :], m[:, 1, :])
                    nc.vector.tensor_add(s2[:], s01[:], m[:, 2, :])
                    nc.vector.tensor_add(
                        out3[:, co : co + cv, h], s2[:], T_sb[:, 3, :]
                    )

            nc.sync.dma_start(out=out_d[:, s0 * 3 : (s0 + sv) * 3], in_=out_t[:])

    _strip_matmul_self_waits(nc)
    if not nc.is_finalized():
        nc.finalize()
    return nc


def _strip_matmul_self_waits(nc):
    fn = nc.m.functions[0]
    pe_sems = set()
    for b in fn.blocks:
        for i in b.instructions:
            if i.opcode == "Matmult":
                for u in i.sync_info.on_update:
                    if u.ant_name.startswith("PE"):
                        pe_sems.add(u.ant_name)
    for b in fn.blocks:
        for i in b.instructions:
            if i.opcode != "Matmult":
                continue
            si = i.sync_info
            kept = [w for w in si.on_wait if w.ant_name not in pe_sems]
            if len(kept) != len(si.on_wait):
                si.on_wait = kept
                i.sync_info = si



_BUILT = {}


def _get_nc():
    if "nc" not in _BUILT:
        _BUILT["nc"] = build_nc()
    return _BUILT["nc"]


def make_in_maps(inputs):
    A34, PF = host_prep(inputs)
    vs = np.ascontiguousarray(
        np.asarray(inputs["v_shaped_expressed"], np.float32).reshape(B, V * 3)
    )
    W = np.asarray(inputs["lbs_weights"], np.float32)
    pd = np.asarray(inputs["posedirs"], np.float32)
    Wt = np.ascontiguousarray(W.T)
    PDt = np.ascontiguousarray(pd.transpose(1, 0, 2).reshape(36, V * 3))
    PFt = np.ascontiguousarray(PF.T)

    in_maps = []
    for c in range(NCORES):
        sl = slice(c * BC, (c + 1) * BC)
        AT_c = A34[sl].transpose(1, 2, 3, 0).reshape(5, 12 * BC)
        pad5 = np.zeros((5, PAD), np.float32)
        pad36 = np.zeros((36, PAD), np.float32)
        wat = np.ascontiguousarray(np.concatenate([Wt, pad5, AT_c], axis=1))
        pfpd = np.ascontiguousarray(
            np.concatenate([PFt[:, sl], PDt, pad36], axis=1)
        )
        in_maps.append(
            {
                "vs": np.ascontiguousarray(vs[sl]),
                "wat": wat,
                "pfpd": pfpd,
            }
        )
    return in_maps


def run_on_device(inputs, trace=False):
    from concourse.bass_utils import run_bass_kernel_spmd

    nc = _get_nc()
    in_maps = make_in_maps(inputs)
    res = run_bass_kernel_spmd(nc, in_maps, list(range(NCORES)), trace=trace)
    out = np.concatenate([res.results[i]["out"] for i in range(NCORES)], axis=0)
    return out.reshape(B, V, 3).astype(np.float32), res


def kernel(**inputs):
    out, _ = run_on_device(inputs, trace=False)
    return out
